# revision 67
# baseline (speedup 1.0000x reference)
"""VQ codebook (DINONewVq) Trainium2 kernel.

Data-parallel over 8 NeuronCores: z is sharded along flattened batch*spatial
(N = B*H*W = 12544 -> 1568 rows/core, padded to 1664 = 13*128), codebooks are
replicated.

Per (quantizer p, 128-row tile) on device:
  psum = 2*z@cb^T - ||cb||^2 - ||z||^2
         PE: bf16 hi/lo-split dot (zh*ch + zh*cl + zl*ch, error ~1e-7) FIRST so
         the dot accumulates at small magnitude, then ONE rank-4 bf16 bias
         matmul (zn split exactly into 3 bf16 parts + cn truncated to bf16)
         LAST, so the final rounding happens once at |dist|~128 — mirroring
         the reference's fp32 rounding sequence. This makes argmax(psum)
         reproduce the reference argmin(dist) bit-faithfully (2/50176 rows
         differ, from the cn bf16 truncation).
  eu   = exp(2*psum + 2*||z||^2) = exp(s/T)     (ACT, per-partition bias
         cancels the zn term exactly; accum_out -> softmax denominator Z)
  argmax via vector.max + vector.max_index on eu (monotone in psum)
  zq   = indirect-DMA gather of codebook rows by idx
  sse += sum((zq - z)^2)  per partition          (loss partial)
  prob = eu * (1/Z)                               (ACT Copy with per-partition
         scale — keeps the normalize off the DVE, whose max/max_index scans
         are the busiest engine)

All per-tile inputs are preloaded in a handful of big DMAs (this walrus
codegen allows very few semaphore waits per instruction — notably one per
matmul — so every PE operand's RAW dep must collapse onto one DMA timeline).

Outputs re-assembled on host: zq_out (B,C,H,W), q_loss scalar, distance_prob (N, P*K).
"""

import sys

import numpy as np

try:
    import concourse.bass  # noqa: F401
except ImportError:
    sys.path.insert(0, "/opt/trn_rl_repo")

B, C, H, W = 16, 512, 28, 28
P, K, D = 4, 2048, 128
N = B * H * W            # 12544
NCORES = 8
NSH = N // NCORES        # 1568 real rows per core
NT = 13                  # 128-row tiles per quantizer per core
NP = NT * 128            # 1664 padded rows per core
TEMP = 0.5
BETA = 0.25

_CACHE = {}


def _build_nc(
    dump_dist=False, dot_mode="f32", grp_argmax=False, pool_loss=False,
    act_norm=True, pool_recip=False, smp_bufs=4, pr_bufs=3,
):
    import concourse.bass as bass
    import concourse.mybir as mybir
    from concourse import bacc
    from concourse.tile import TileContext
    from concourse.bass import ts

    FD = mybir.dt.float32
    U32 = mybir.dt.uint32
    nc = bacc.Bacc("TRN2", target_bir_lowering=False)

    # All PE SBUF operands with 128 partitions in ONE tensor/DMA so matmuls
    # carry at most one RAW semaphore.
    BF = mybir.dt.bfloat16
    if dot_mode in ("bf16x2", "bf16full"):
        # hi/lo bf16 split: dot = zh*ch + zh*cl + zl*ch (zl*cl ~ 2e-8, dropped)
        # cols [0:PK]=ch, [PK:2PK]=cl, [2PK:2PK+PN]=zh, [2PK+PN:]=zl
        pe_in = nc.dram_tensor(
            "pe_in", [D, 2 * (P * K + P * NP)], BF, kind="ExternalInput"
        )
    else:
        FR = mybir.dt.float32r if dot_mode == "f32r" else mybir.dt.float32
        pe_in = nc.dram_tensor("pe_in", [D, P * K + P * NP], FR, kind="ExternalInput")
    if dot_mode == "bf16full":
        # rank-4 bf16 bias: zn split exactly into 3 bf16 parts, cn kept to
        # bf16-hi only (costs ~1e-2 flip-probability-weighted rows total).
        # per p: cols [0:K] rows = (1 ; 1 ; 1 ; -cn_hi),
        #        cols [K:K+NP] rows = (-zn_s1 ; -zn_s2 ; -zn_s3 ; 1)
        bias2 = nc.dram_tensor("bias2", [P, 4, K + NP], BF, kind="ExternalInput")
    else:
        # per p: cols [0:K] = (-cn ; 1), cols [K:K+NP] = (1 ; -zn)
        bias2 = nc.dram_tensor("bias2", [P, 2, K + NP], FD, kind="ExternalInput")
    zf_n = nc.dram_tensor("zf_n", [P, NP, D], FD, kind="ExternalInput")
    zn2 = nc.dram_tensor("zn2", [P, 128, NT], FD, kind="ExternalInput")
    cbr = nc.dram_tensor("cbr", [P * K, D], FD, kind="ExternalInput")

    prob = nc.dram_tensor("prob", [P, NP, K], FD, kind="ExternalOutput")
    zq = nc.dram_tensor("zq", [P, NP, D], FD, kind="ExternalOutput")
    sse = nc.dram_tensor("sse", [128, P * NT], FD, kind="ExternalOutput")
    dist = (
        nc.dram_tensor("dist", [P, NP, K], FD, kind="ExternalOutput")
        if dump_dist
        else None
    )

    with TileContext(nc) as tc:
        with (
            tc.tile_pool(name="const", bufs=1) as cpool,
            tc.tile_pool(name="io", bufs=3) as iop,
            tc.tile_pool(name="mm", bufs=2, space="PSUM") as psp,
            tc.tile_pool(name="eu", bufs=3) as eup,
            tc.tile_pool(name="pr", bufs=pr_bufs) as prp,
            tc.tile_pool(name="sm", bufs=smp_bufs) as smp,
        ):
            # fp32 self-loading Matmult (and direct2d DMA) tolerate very few
            # sync waits in this walrus codegen; keep every instruction's deps
            # on as few semaphore timelines as possible by preloading ALL
            # per-tile inputs with a handful of big DMAs.
            if dot_mode in ("bf16x2", "bf16full"):
                pe_sb = cpool.tile(
                    [D, 2 * (P * K + P * NP)], mybir.dt.bfloat16, tag="pe_in"
                )
            else:
                pe_sb = cpool.tile([D, P * K + P * NP], FR, tag="pe_in")
            if dot_mode == "bf16full":
                bias2_sb = cpool.tile([4, P * (K + NP)], BF, tag="bias2")
            else:
                bias2_sb = cpool.tile([2, P * (K + NP)], FD, tag="bias2")
            zfn_sb = cpool.tile([128, P * NT * D], FD, tag="zfn")
            zn2_sb = cpool.tile([128, P * NT], FD, tag="zn2")
            sse_sb = cpool.tile([128, P * NT], FD, tag="sse")
            ones_c = cpool.tile([128, 1], FD, tag="ones_c")
            nc.vector.memset(ones_c, 1.0)
            iota16 = cpool.tile([128, 16], FD, tag="iota16")
            nc.gpsimd.iota(
                iota16,
                pattern=[[1, 16]],
                base=0,
                channel_multiplier=0,
                allow_small_or_imprecise_dtypes=True,
            )
            nc.sync.dma_start(pe_sb, pe_in[:])
            nc.sync.dma_start(
                bias2_sb[:].rearrange("r (p c) -> r p c", p=P),
                bias2[:].rearrange("p r c -> r p c"),
            )
            nc.sync.dma_start(
                zfn_sb[:].rearrange("r (p t d) -> r p t d", p=P, t=NT),
                zf_n[:].rearrange("p (t r) d -> r p t d", r=128),
            )
            nc.sync.dma_start(
                zn2_sb[:].rearrange("r (p t) -> r p t", p=P),
                zn2[:].rearrange("p r t -> r p t"),
            )

            PK = P * K
            for p in range(P):
                for i in range(NT):
                    col = p * NT + i
                    zfn = zfn_sb[:, col * D : (col + 1) * D]

                    ps = psp.tile([128, K], FD, tag="ps")
                    pb = p * (K + NP)
                    l2 = bias2_sb[:, pb + K + i * 128 : pb + K + (i + 1) * 128]
                    # Dot FIRST (psum stays small-magnitude so the split
                    # accumulation keeps the low dot bits), bias rank-2 LAST
                    # (single rounding at |dist| ~ 128) — mirrors the
                    # reference's fp32 rounding sequence so argmin matches.
                    if dot_mode in ("bf16x2", "bf16full"):
                        zh = pe_sb[
                            :, 2 * PK + p * NP + i * 128 : 2 * PK + p * NP + (i + 1) * 128
                        ]
                        zl = pe_sb[
                            :,
                            2 * PK + P * NP + p * NP + i * 128 : 2 * PK
                            + P * NP
                            + p * NP
                            + (i + 1) * 128,
                        ]
                        for j in range(4):
                            ch = pe_sb[:, p * K + j * 512 : p * K + (j + 1) * 512]
                            cl = pe_sb[
                                :, PK + p * K + j * 512 : PK + p * K + (j + 1) * 512
                            ]
                            nc.tensor.matmul(
                                ps[:, ts(j, 512)], lhsT=zh, rhs=ch,
                                start=True, stop=False,
                            )
                            nc.tensor.matmul(
                                ps[:, ts(j, 512)], lhsT=zh, rhs=cl,
                                start=False, stop=False,
                            )
                            nc.tensor.matmul(
                                ps[:, ts(j, 512)], lhsT=zl, rhs=ch,
                                start=False, stop=False,
                            )
                    else:
                        zfw = pe_sb[
                            :, PK + p * NP + i * 128 : PK + p * NP + (i + 1) * 128
                        ]
                        for j in range(4):
                            nc.tensor.matmul(
                                ps[:, ts(j, 512)],
                                lhsT=zfw,
                                rhs=pe_sb[:, p * K + j * 512 : p * K + (j + 1) * 512],
                                start=True,
                                stop=False,
                            )
                    for j in range(4):
                        nc.tensor.matmul(
                            ps[:, ts(j, 512)],
                            lhsT=l2,
                            rhs=bias2_sb[:, pb + j * 512 : pb + (j + 1) * 512],
                            start=False,
                            stop=True,
                        )

                    if dist is not None:
                        dd = prp.tile([128, K], FD, tag="pr")
                        nc.vector.tensor_copy(dd, ps)
                        nc.sync.dma_start(dist[p, ts(i, 128), :], dd)
                    eu = eup.tile([128, K], FD, tag="eu")
                    z_i = smp.tile([128, 1], FD, tag="z")
                    nc.scalar.activation(
                        out=eu,
                        in_=ps,
                        func=mybir.ActivationFunctionType.Exp,
                        scale=2.0,
                        bias=zn2_sb[:, col : col + 1],
                        accum_out=z_i,
                    )

                    if pool_recip:
                        # normalize_recip on a 1-wide dummy overwrites z_i
                        # with 1/Z on the (otherwise idle) Pool engine
                        dum = smp.tile([128, 1], FD, tag="dum")
                        nc.gpsimd.normalize_recip(
                            out_ap=dum, in_ap=ones_c, denom_ap=z_i
                        )
                        rz = z_i
                    else:
                        rz = smp.tile([128, 1], FD, tag="rz")
                        nc.vector.reciprocal(rz, z_i)
                    pr = prp.tile([128, K], FD, tag="pr")
                    if act_norm:
                        nc.scalar.activation(
                            out=pr,
                            in_=eu,
                            func=mybir.ActivationFunctionType.Copy,
                            scale=rz,
                        )
                    else:
                        nc.vector.tensor_scalar_mul(pr, eu, rz)
                    nc.sync.dma_start(prob[p, ts(i, 128), :], pr)

                    if grp_argmax:
                        # two-level argmax on prob (monotone in the grid dist):
                        # group-max over 16-wide groups, locate group, DGE-gather
                        # the 16 candidates back from DRAM, locate position.
                        sm = smp.tile([128, 128], FD, tag="sm")
                        nc.vector.tensor_reduce(
                            sm,
                            pr[:].rearrange("r (g s) -> r g s", s=16),
                            mybir.AxisListType.X,
                            mybir.AluOpType.max,
                        )
                        m8 = smp.tile([128, 8], FD, tag="m8")
                        nc.vector.max(out=m8, in_=sm)
                        g8 = smp.tile([128, 8], U32, tag="g8")
                        nc.vector.max_index(out=g8, in_max=m8, in_values=sm)
                        g_f = smp.tile([128, 1], FD, tag="g_f")
                        nc.vector.tensor_copy(g_f, g8[:, 0:1])
                        g16_f = smp.tile([128, 1], FD, tag="g16_f")
                        nc.vector.tensor_scalar(
                            out=g16_f,
                            in0=g_f,
                            scalar1=16.0,
                            scalar2=None,
                            op0=mybir.AluOpType.mult,
                        )
                        # per-partition in-SBUF gather of the winning 16-group
                        gidx = smp.tile([128, 16], mybir.dt.uint16, tag="gidx")
                        nc.vector.tensor_scalar(
                            out=gidx,
                            in0=iota16,
                            scalar1=g16_f,
                            scalar2=None,
                            op0=mybir.AluOpType.add,
                        )
                        grp = smp.tile([128, 16], FD, tag="grp")
                        nc.gpsimd.indirect_copy(
                            out=grp, data=pr, idxs=gidx,
                            i_know_ap_gather_is_preferred=True,
                        )
                        p8 = smp.tile([128, 8], U32, tag="p8")
                        nc.vector.max_index(out=p8, in_max=m8, in_values=grp)
                        p_f = smp.tile([128, 1], FD, tag="p_f")
                        nc.vector.tensor_copy(p_f, p8[:, 0:1])
                        t1f = smp.tile([128, 1], FD, tag="t1f")
                        nc.vector.tensor_scalar(
                            out=t1f,
                            in0=g16_f,
                            scalar1=float(p * K),
                            scalar2=None,
                            op0=mybir.AluOpType.add,
                        )
                        idxa = smp.tile([128, 1], U32, tag="idxa")
                        nc.vector.tensor_add(idxa, t1f, p_f)
                    else:
                        m8 = smp.tile([128, 8], FD, tag="m8")
                        nc.vector.max(out=m8, in_=eu)
                        i8 = smp.tile([128, 8], U32, tag="i8")
                        nc.vector.max_index(out=i8, in_max=m8, in_values=eu)
                        idxa = smp.tile([128, 1], U32, tag="idxa")
                        nc.vector.tensor_scalar_add(idxa, i8[:, 0:1], p * K)

                    zq_sb = iop.tile([128, D], FD, tag="zq")
                    nc.gpsimd.indirect_dma_start(
                        out=zq_sb,
                        out_offset=None,
                        in_=cbr[:],
                        in_offset=bass.IndirectOffsetOnAxis(ap=idxa, axis=0),
                    )
                    nc.sync.dma_start(zq[p, ts(i, 128), :], zq_sb)

                    if pool_loss:
                        diff = iop.tile([128, D], FD, tag="diff")
                        nc.gpsimd.tensor_sub(diff, zq_sb, zfn)
                        sq = iop.tile([128, D], FD, tag="sq")
                        nc.gpsimd.tensor_mul(sq, diff, diff)
                        nc.vector.tensor_reduce(
                            sse_sb[:, col : col + 1],
                            sq,
                            mybir.AxisListType.X,
                            mybir.AluOpType.add,
                        )
                    else:
                        diff = iop.tile([128, D], FD, tag="diff")
                        nc.vector.tensor_sub(diff, zq_sb, zfn)
                        sq = iop.tile([128, D], FD, tag="sq")
                        nc.vector.scalar_tensor_tensor(
                            out=sq,
                            in0=diff,
                            scalar=1.0,
                            in1=diff,
                            op0=mybir.AluOpType.mult,
                            op1=mybir.AluOpType.mult,
                            accum_out=sse_sb[:, col : col + 1],
                        )

            nc.sync.dma_start(sse[:, :], sse_sb)

    nc.compile()
    return nc


def _prep_inputs(z, codebooks, dot_mode="f32"):
    import ml_dtypes

    bf16 = np.dtype(ml_dtypes.bfloat16)
    z = np.ascontiguousarray(np.asarray(z, dtype=np.float32))
    cb = np.ascontiguousarray(np.asarray(codebooks, dtype=np.float32))

    # (B, C, H, W) -> (P, N, d) with n = b*(H*W) + hw
    zf = np.transpose(z.reshape(B, P, D, H * W), (1, 0, 3, 2)).reshape(P, N, D)
    zn = np.sum(zf * zf, axis=-1)                       # (P, N)
    cn = np.sum(cb * cb, axis=-1)                       # (P, K)
    cbT = np.transpose(cb, (0, 2, 1))                   # (P, d, K)

    cb2 = np.ascontiguousarray(2.0 * cbT)
    cbr = np.ascontiguousarray(cb.reshape(P * K, D))

    in_maps = []
    for c in range(NCORES):
        lo = c * NSH
        zf_s = np.zeros((P, NP, D), np.float32)
        zf_s[:, :NSH] = zf[:, lo : lo + NSH]
        zn_s = np.zeros((P, NP), np.float32)
        zn_s[:, :NSH] = zn[:, lo : lo + NSH]

        cbt = cb2.transpose(1, 0, 2).reshape(D, P * K)          # 2*cb^T, d-major
        zft = zf_s.transpose(2, 0, 1).reshape(D, P * NP)        # zf^T, d-major
        if dot_mode in ("bf16x2", "bf16full"):
            ch = cbt.astype(bf16)
            cl = (cbt - ch.astype(np.float32)).astype(bf16)
            zh = zft.astype(bf16)
            zl = (zft - zh.astype(np.float32)).astype(bf16)
            pe_in = np.ascontiguousarray(np.concatenate([ch, cl, zh, zl], axis=1))
        else:
            pe_in = np.ascontiguousarray(
                np.concatenate([cbt, zft], axis=1), dtype=np.float32
            )

        if dot_mode == "bf16full":
            # exact 3-way bf16 split of zn; cn truncated to bf16 hi part
            s1 = zn_s.astype(bf16)
            r1 = zn_s - s1.astype(np.float32)
            s2 = r1.astype(bf16)
            r2 = r1 - s2.astype(np.float32)
            s3 = r2.astype(bf16)
            assert float(np.abs(
                zn_s - (s1.astype(np.float32) + s2.astype(np.float32)
                        + s3.astype(np.float32))
            ).max()) == 0.0, "zn bf16 3-split not exact"
            bias2 = np.zeros((P, 4, K + NP), bf16)
            bias2[:, 0, :K] = 1.0
            bias2[:, 1, :K] = 1.0
            bias2[:, 2, :K] = 1.0
            bias2[:, 3, :K] = (-cn).astype(bf16)
            bias2[:, 0, K:] = -s1
            bias2[:, 1, K:] = -s2
            bias2[:, 2, K:] = -s3
            bias2[:, 3, K:] = 1.0
        else:
            bias2 = np.empty((P, 2, K + NP), np.float32)
            bias2[:, 0, :K] = -cn
            bias2[:, 1, :K] = 1.0
            bias2[:, 0, K:] = 1.0
            bias2[:, 1, K:] = -zn_s

        in_maps.append(
            {
                "pe_in": pe_in,
                "zf_n": zf_s,
                "bias2": bias2,
                "zn2": np.ascontiguousarray(
                    (2.0 * zn_s).reshape(P, NT, 128).transpose(0, 2, 1)
                ),
                "cbr": cbr,
            }
        )
    return in_maps, zf, cb


DOT_MODE = "bf16full"


def kernel(z, codebooks):
    from concourse.bass_utils import run_bass_kernel_spmd

    if "nc" not in _CACHE:
        _CACHE["nc"] = _build_nc(dot_mode=DOT_MODE)
    nc = _CACHE["nc"]

    in_maps, zf, cb = _prep_inputs(z, codebooks, dot_mode=DOT_MODE)
    res = run_bass_kernel_spmd(nc, in_maps, core_ids=list(range(NCORES)))
    outs = res.results

    # --- distance_prob: (N, P*K) ---
    distance_prob = np.empty((N, P * K), np.float32)
    zq_pnd = np.empty((P, N, D), np.float32)
    sse_total = 0.0
    for c in range(NCORES):
        lo = c * NSH
        o = outs[c]
        pr = o["prob"]                                # (P, NP, K)
        zqc = o["zq"]                                 # (P, NP, D)
        for p in range(P):
            distance_prob[lo : lo + NSH, p * K : (p + 1) * K] = pr[p, :NSH]
        zq_pnd[:, lo : lo + NSH] = zqc[:, :NSH]
        s = o["sse"].astype(np.float64)               # (128, P*NT)
        # tail tile (i == NT-1) only has rows NSH - 12*128 = 32 valid
        valid_tail = NSH - (NT - 1) * 128
        for p in range(P):
            sse_total += s[:, p * NT : p * NT + NT - 1].sum()
            sse_total += s[:valid_tail, p * NT + NT - 1].sum()

    q_loss = np.float32((1.0 + BETA) * sse_total / (P * N * D))

    # --- zq back to (B, C, H, W) ---
    zq_out = (
        zq_pnd.reshape(P, B, H * W, D)
        .transpose(1, 0, 3, 2)
        .reshape(B, C, H, W)
    )
    return np.ascontiguousarray(zq_out), q_loss, distance_prob


# revision 70
# speedup vs baseline: 1.0236x; 1.0236x over previous
"""VQ codebook (DINONewVq) Trainium2 kernel.

Data-parallel over 8 NeuronCores: z is sharded along flattened batch*spatial
(N = B*H*W = 12544 -> 1568 rows/core, padded to 1664 = 13*128), codebooks are
replicated.

Per (quantizer p, 128-row tile) on device:
  psum = 2*z@cb^T - ||cb||^2 - ||z||^2
         PE: bf16 hi/lo-split dot (zh*ch + zh*cl + zl*ch, error ~1e-7) FIRST so
         the dot accumulates at small magnitude, then ONE rank-4 bf16 bias
         matmul (zn split exactly into 3 bf16 parts + cn truncated to bf16)
         LAST, so the final rounding happens once at |dist|~128 — mirroring
         the reference's fp32 rounding sequence. This makes argmax(psum)
         reproduce the reference argmin(dist) bit-faithfully (2/50176 rows
         differ, from the cn bf16 truncation).
  eu   = exp(2*psum + 2*||z||^2) = exp(s/T)     (ACT, per-partition bias
         cancels the zn term exactly; accum_out -> softmax denominator Z)
  argmax via vector.max + vector.max_index on eu (monotone in psum)
  zq   = indirect-DMA gather of codebook rows by idx
  sse += sum((zq - z)^2)  per partition          (loss partial)
  prob = eu * (1/Z)                               (ACT Copy with per-partition
         scale — keeps the normalize off the DVE, whose max/max_index scans
         are the busiest engine)

All per-tile inputs are preloaded in a handful of big DMAs (this walrus
codegen allows very few semaphore waits per instruction — notably one per
matmul — so every PE operand's RAW dep must collapse onto one DMA timeline).

Outputs re-assembled on host: zq_out (B,C,H,W), q_loss scalar, distance_prob (N, P*K).
"""

import sys

import numpy as np

try:
    import concourse.bass  # noqa: F401
except ImportError:
    sys.path.insert(0, "/opt/trn_rl_repo")

B, C, H, W = 16, 512, 28, 28
P, K, D = 4, 2048, 128
N = B * H * W            # 12544
NCORES = 8
NSH = N // NCORES        # 1568 real rows per core
NT = 13                  # 128-row tiles per quantizer per core
NP = NT * 128            # 1664 padded rows per core
TEMP = 0.5
BETA = 0.25

_CACHE = {}


def _build_nc(
    dump_dist=False, dot_mode="f32", grp_argmax=False, pool_loss=False,
    act_norm=True, pool_recip=False, smp_bufs=4, pr_bufs=3, split_preload=False,
):
    import concourse.bass as bass
    import concourse.mybir as mybir
    from concourse import bacc
    from concourse.tile import TileContext
    from concourse.bass import ts

    FD = mybir.dt.float32
    U32 = mybir.dt.uint32
    nc = bacc.Bacc("TRN2", target_bir_lowering=False)

    # All PE SBUF operands with 128 partitions in ONE tensor/DMA so matmuls
    # carry at most one RAW semaphore.
    BF = mybir.dt.bfloat16
    if dot_mode in ("bf16x2", "bf16full"):
        # hi/lo bf16 split: dot = zh*ch + zh*cl + zl*ch (zl*cl ~ 2e-8, dropped)
        # cols [0:PK]=ch, [PK:2PK]=cl, [2PK:2PK+PN]=zh, [2PK+PN:]=zl
        if split_preload:
            pe_in = nc.dram_tensor(
                "pe_in", [P, D, 2 * (K + NP)], BF, kind="ExternalInput"
            )
        else:
            pe_in = nc.dram_tensor(
                "pe_in", [D, 2 * (P * K + P * NP)], BF, kind="ExternalInput"
            )
    else:
        FR = mybir.dt.float32r if dot_mode == "f32r" else mybir.dt.float32
        pe_in = nc.dram_tensor("pe_in", [D, P * K + P * NP], FR, kind="ExternalInput")
    if dot_mode == "bf16full":
        # rank-4 bf16 bias: zn split exactly into 3 bf16 parts, cn kept to
        # bf16-hi only (costs ~1e-2 flip-probability-weighted rows total).
        # per p: cols [0:K] rows = (1 ; 1 ; 1 ; -cn_hi),
        #        cols [K:K+NP] rows = (-zn_s1 ; -zn_s2 ; -zn_s3 ; 1)
        bias2 = nc.dram_tensor("bias2", [P, 4, K + NP], BF, kind="ExternalInput")
    else:
        # per p: cols [0:K] = (-cn ; 1), cols [K:K+NP] = (1 ; -zn)
        bias2 = nc.dram_tensor("bias2", [P, 2, K + NP], FD, kind="ExternalInput")
    zf_n = nc.dram_tensor("zf_n", [P, NP, D], FD, kind="ExternalInput")
    zn2 = nc.dram_tensor("zn2", [P, 128, NT], FD, kind="ExternalInput")
    cbr = nc.dram_tensor("cbr", [P * K, D], FD, kind="ExternalInput")

    prob = nc.dram_tensor("prob", [P, NP, K], FD, kind="ExternalOutput")
    zq = nc.dram_tensor("zq", [P, NP, D], FD, kind="ExternalOutput")
    sse = nc.dram_tensor("sse", [128, P * NT], FD, kind="ExternalOutput")
    dist = (
        nc.dram_tensor("dist", [P, NP, K], FD, kind="ExternalOutput")
        if dump_dist
        else None
    )

    with TileContext(nc) as tc:
        with (
            tc.tile_pool(name="const", bufs=1) as cpool,
            tc.tile_pool(name="io", bufs=3) as iop,
            tc.tile_pool(name="mm", bufs=2, space="PSUM") as psp,
            tc.tile_pool(name="eu", bufs=3) as eup,
            tc.tile_pool(name="pr", bufs=pr_bufs) as prp,
            tc.tile_pool(name="sm", bufs=smp_bufs) as smp,
        ):
            # fp32 self-loading Matmult (and direct2d DMA) tolerate very few
            # sync waits in this walrus codegen; keep every instruction's deps
            # on as few semaphore timelines as possible by preloading ALL
            # per-tile inputs with a handful of big DMAs.
            if dot_mode in ("bf16x2", "bf16full") and split_preload:
                pe_ps = []
                for q in range(P):
                    pe_q = cpool.tile(
                        [D, 2 * (K + NP)], mybir.dt.bfloat16, tag=f"pe{q}"
                    )
                    pe_ps.append(pe_q)
            elif dot_mode in ("bf16x2", "bf16full"):
                pe_sb = cpool.tile(
                    [D, 2 * (P * K + P * NP)], mybir.dt.bfloat16, tag="pe_in"
                )
            else:
                pe_sb = cpool.tile([D, P * K + P * NP], FR, tag="pe_in")
            if dot_mode == "bf16full":
                bias2_sb = cpool.tile([4, P * (K + NP)], BF, tag="bias2")
            else:
                bias2_sb = cpool.tile([2, P * (K + NP)], FD, tag="bias2")
            zfn_sb = cpool.tile([128, P * NT * D], FD, tag="zfn")
            zn2_sb = cpool.tile([128, P * NT], FD, tag="zn2")
            sse_sb = cpool.tile([128, P * NT], FD, tag="sse")
            ones_c = cpool.tile([128, 1], FD, tag="ones_c")
            nc.vector.memset(ones_c, 1.0)
            iota16 = cpool.tile([128, 16], FD, tag="iota16")
            nc.gpsimd.iota(
                iota16,
                pattern=[[1, 16]],
                base=0,
                channel_multiplier=0,
                allow_small_or_imprecise_dtypes=True,
            )
            if split_preload:
                # need-ordered: p=0 weights, tiny bias/zn (tile-0 critical
                # path), remaining p's weights, and the loss operand last
                nc.sync.dma_start(pe_ps[0], pe_in[0])
                nc.sync.dma_start(
                    bias2_sb[:].rearrange("r (p c) -> r p c", p=P),
                    bias2[:].rearrange("p r c -> r p c"),
                )
                nc.sync.dma_start(
                    zn2_sb[:].rearrange("r (p t) -> r p t", p=P),
                    zn2[:].rearrange("p r t -> r p t"),
                )
                for q in range(1, P):
                    nc.sync.dma_start(pe_ps[q], pe_in[q])
                nc.sync.dma_start(
                    zfn_sb[:].rearrange("r (p t d) -> r p t d", p=P, t=NT),
                    zf_n[:].rearrange("p (t r) d -> r p t d", r=128),
                )
            else:
                nc.sync.dma_start(pe_sb, pe_in[:])
                nc.sync.dma_start(
                    bias2_sb[:].rearrange("r (p c) -> r p c", p=P),
                    bias2[:].rearrange("p r c -> r p c"),
                )
                nc.sync.dma_start(
                    zfn_sb[:].rearrange("r (p t d) -> r p t d", p=P, t=NT),
                    zf_n[:].rearrange("p (t r) d -> r p t d", r=128),
                )
                nc.sync.dma_start(
                    zn2_sb[:].rearrange("r (p t) -> r p t", p=P),
                    zn2[:].rearrange("p r t -> r p t"),
                )

            PK = P * K
            for p in range(P):
                for i in range(NT):
                    col = p * NT + i
                    zfn = zfn_sb[:, col * D : (col + 1) * D]

                    ps = psp.tile([128, K], FD, tag="ps")
                    pb = p * (K + NP)
                    l2 = bias2_sb[:, pb + K + i * 128 : pb + K + (i + 1) * 128]
                    # Dot FIRST (psum stays small-magnitude so the split
                    # accumulation keeps the low dot bits), bias rank-2 LAST
                    # (single rounding at |dist| ~ 128) — mirrors the
                    # reference's fp32 rounding sequence so argmin matches.
                    if dot_mode in ("bf16x2", "bf16full") and split_preload:
                        pe_p = pe_ps[p]
                        zh = pe_p[:, 2 * K + i * 128 : 2 * K + (i + 1) * 128]
                        zl = pe_p[
                            :, 2 * K + NP + i * 128 : 2 * K + NP + (i + 1) * 128
                        ]
                        for j in range(4):
                            ch = pe_p[:, j * 512 : (j + 1) * 512]
                            cl = pe_p[:, K + j * 512 : K + (j + 1) * 512]
                            nc.tensor.matmul(
                                ps[:, ts(j, 512)], lhsT=zh, rhs=ch,
                                start=True, stop=False,
                            )
                            nc.tensor.matmul(
                                ps[:, ts(j, 512)], lhsT=zh, rhs=cl,
                                start=False, stop=False,
                            )
                            nc.tensor.matmul(
                                ps[:, ts(j, 512)], lhsT=zl, rhs=ch,
                                start=False, stop=False,
                            )
                    elif dot_mode in ("bf16x2", "bf16full"):
                        zh = pe_sb[
                            :, 2 * PK + p * NP + i * 128 : 2 * PK + p * NP + (i + 1) * 128
                        ]
                        zl = pe_sb[
                            :,
                            2 * PK + P * NP + p * NP + i * 128 : 2 * PK
                            + P * NP
                            + p * NP
                            + (i + 1) * 128,
                        ]
                        for j in range(4):
                            ch = pe_sb[:, p * K + j * 512 : p * K + (j + 1) * 512]
                            cl = pe_sb[
                                :, PK + p * K + j * 512 : PK + p * K + (j + 1) * 512
                            ]
                            nc.tensor.matmul(
                                ps[:, ts(j, 512)], lhsT=zh, rhs=ch,
                                start=True, stop=False,
                            )
                            nc.tensor.matmul(
                                ps[:, ts(j, 512)], lhsT=zh, rhs=cl,
                                start=False, stop=False,
                            )
                            nc.tensor.matmul(
                                ps[:, ts(j, 512)], lhsT=zl, rhs=ch,
                                start=False, stop=False,
                            )
                    else:
                        zfw = pe_sb[
                            :, PK + p * NP + i * 128 : PK + p * NP + (i + 1) * 128
                        ]
                        for j in range(4):
                            nc.tensor.matmul(
                                ps[:, ts(j, 512)],
                                lhsT=zfw,
                                rhs=pe_sb[:, p * K + j * 512 : p * K + (j + 1) * 512],
                                start=True,
                                stop=False,
                            )
                    for j in range(4):
                        nc.tensor.matmul(
                            ps[:, ts(j, 512)],
                            lhsT=l2,
                            rhs=bias2_sb[:, pb + j * 512 : pb + (j + 1) * 512],
                            start=False,
                            stop=True,
                        )

                    if dist is not None:
                        dd = prp.tile([128, K], FD, tag="pr")
                        nc.vector.tensor_copy(dd, ps)
                        nc.sync.dma_start(dist[p, ts(i, 128), :], dd)
                    eu = eup.tile([128, K], FD, tag="eu")
                    z_i = smp.tile([128, 1], FD, tag="z")
                    nc.scalar.activation(
                        out=eu,
                        in_=ps,
                        func=mybir.ActivationFunctionType.Exp,
                        scale=2.0,
                        bias=zn2_sb[:, col : col + 1],
                        accum_out=z_i,
                    )

                    if pool_recip:
                        # normalize_recip on a 1-wide dummy overwrites z_i
                        # with 1/Z on the (otherwise idle) Pool engine
                        dum = smp.tile([128, 1], FD, tag="dum")
                        nc.gpsimd.normalize_recip(
                            out_ap=dum, in_ap=ones_c, denom_ap=z_i
                        )
                        rz = z_i
                    else:
                        rz = smp.tile([128, 1], FD, tag="rz")
                        nc.vector.reciprocal(rz, z_i)
                    pr = prp.tile([128, K], FD, tag="pr")
                    if act_norm:
                        nc.scalar.activation(
                            out=pr,
                            in_=eu,
                            func=mybir.ActivationFunctionType.Copy,
                            scale=rz,
                        )
                    else:
                        nc.vector.tensor_scalar_mul(pr, eu, rz)
                    nc.sync.dma_start(prob[p, ts(i, 128), :], pr)

                    if grp_argmax:
                        # two-level argmax on prob (monotone in the grid dist):
                        # group-max over 16-wide groups, locate group, DGE-gather
                        # the 16 candidates back from DRAM, locate position.
                        sm = smp.tile([128, 128], FD, tag="sm")
                        nc.vector.tensor_reduce(
                            sm,
                            pr[:].rearrange("r (g s) -> r g s", s=16),
                            mybir.AxisListType.X,
                            mybir.AluOpType.max,
                        )
                        m8 = smp.tile([128, 8], FD, tag="m8")
                        nc.vector.max(out=m8, in_=sm)
                        g8 = smp.tile([128, 8], U32, tag="g8")
                        nc.vector.max_index(out=g8, in_max=m8, in_values=sm)
                        g_f = smp.tile([128, 1], FD, tag="g_f")
                        nc.vector.tensor_copy(g_f, g8[:, 0:1])
                        g16_f = smp.tile([128, 1], FD, tag="g16_f")
                        nc.vector.tensor_scalar(
                            out=g16_f,
                            in0=g_f,
                            scalar1=16.0,
                            scalar2=None,
                            op0=mybir.AluOpType.mult,
                        )
                        # per-partition in-SBUF gather of the winning 16-group
                        gidx = smp.tile([128, 16], mybir.dt.uint16, tag="gidx")
                        nc.vector.tensor_scalar(
                            out=gidx,
                            in0=iota16,
                            scalar1=g16_f,
                            scalar2=None,
                            op0=mybir.AluOpType.add,
                        )
                        grp = smp.tile([128, 16], FD, tag="grp")
                        nc.gpsimd.indirect_copy(
                            out=grp, data=pr, idxs=gidx,
                            i_know_ap_gather_is_preferred=True,
                        )
                        p8 = smp.tile([128, 8], U32, tag="p8")
                        nc.vector.max_index(out=p8, in_max=m8, in_values=grp)
                        p_f = smp.tile([128, 1], FD, tag="p_f")
                        nc.vector.tensor_copy(p_f, p8[:, 0:1])
                        t1f = smp.tile([128, 1], FD, tag="t1f")
                        nc.vector.tensor_scalar(
                            out=t1f,
                            in0=g16_f,
                            scalar1=float(p * K),
                            scalar2=None,
                            op0=mybir.AluOpType.add,
                        )
                        idxa = smp.tile([128, 1], U32, tag="idxa")
                        nc.vector.tensor_add(idxa, t1f, p_f)
                    else:
                        m8 = smp.tile([128, 8], FD, tag="m8")
                        nc.vector.max(out=m8, in_=eu)
                        i8 = smp.tile([128, 8], U32, tag="i8")
                        nc.vector.max_index(out=i8, in_max=m8, in_values=eu)
                        idxa = smp.tile([128, 1], U32, tag="idxa")
                        nc.vector.tensor_scalar_add(idxa, i8[:, 0:1], p * K)

                    zq_sb = iop.tile([128, D], FD, tag="zq")
                    nc.gpsimd.indirect_dma_start(
                        out=zq_sb,
                        out_offset=None,
                        in_=cbr[:],
                        in_offset=bass.IndirectOffsetOnAxis(ap=idxa, axis=0),
                    )
                    nc.sync.dma_start(zq[p, ts(i, 128), :], zq_sb)

                    if pool_loss == "sub":
                        diff = iop.tile([128, D], FD, tag="diff")
                        nc.gpsimd.tensor_sub(diff, zq_sb, zfn)
                        sq = iop.tile([128, D], FD, tag="sq")
                        nc.vector.scalar_tensor_tensor(
                            out=sq,
                            in0=diff,
                            scalar=1.0,
                            in1=diff,
                            op0=mybir.AluOpType.mult,
                            op1=mybir.AluOpType.mult,
                            accum_out=sse_sb[:, col : col + 1],
                        )
                    elif pool_loss:
                        diff = iop.tile([128, D], FD, tag="diff")
                        nc.gpsimd.tensor_sub(diff, zq_sb, zfn)
                        sq = iop.tile([128, D], FD, tag="sq")
                        nc.gpsimd.tensor_mul(sq, diff, diff)
                        nc.vector.tensor_reduce(
                            sse_sb[:, col : col + 1],
                            sq,
                            mybir.AxisListType.X,
                            mybir.AluOpType.add,
                        )
                    else:
                        diff = iop.tile([128, D], FD, tag="diff")
                        nc.vector.tensor_sub(diff, zq_sb, zfn)
                        sq = iop.tile([128, D], FD, tag="sq")
                        nc.vector.scalar_tensor_tensor(
                            out=sq,
                            in0=diff,
                            scalar=1.0,
                            in1=diff,
                            op0=mybir.AluOpType.mult,
                            op1=mybir.AluOpType.mult,
                            accum_out=sse_sb[:, col : col + 1],
                        )

            nc.sync.dma_start(sse[:, :], sse_sb)

    nc.compile()
    return nc


def _prep_inputs(z, codebooks, dot_mode="f32"):
    import ml_dtypes

    bf16 = np.dtype(ml_dtypes.bfloat16)
    z = np.ascontiguousarray(np.asarray(z, dtype=np.float32))
    cb = np.ascontiguousarray(np.asarray(codebooks, dtype=np.float32))

    # (B, C, H, W) -> (P, N, d) with n = b*(H*W) + hw
    zf = np.transpose(z.reshape(B, P, D, H * W), (1, 0, 3, 2)).reshape(P, N, D)
    zn = np.sum(zf * zf, axis=-1)                       # (P, N)
    cn = np.sum(cb * cb, axis=-1)                       # (P, K)
    cbT = np.transpose(cb, (0, 2, 1))                   # (P, d, K)

    cb2 = np.ascontiguousarray(2.0 * cbT)
    cbr = np.ascontiguousarray(cb.reshape(P * K, D))

    in_maps = []
    for c in range(NCORES):
        lo = c * NSH
        zf_s = np.zeros((P, NP, D), np.float32)
        zf_s[:, :NSH] = zf[:, lo : lo + NSH]
        zn_s = np.zeros((P, NP), np.float32)
        zn_s[:, :NSH] = zn[:, lo : lo + NSH]

        cbt = cb2.transpose(1, 0, 2).reshape(D, P * K)          # 2*cb^T, d-major
        zft = zf_s.transpose(2, 0, 1).reshape(D, P * NP)        # zf^T, d-major
        if dot_mode in ("bf16x2", "bf16full") and SPLIT_PRELOAD:
            cbp = cb2                                           # (P, D, K)
            zfp = zf_s.transpose(0, 2, 1)                       # (P, D, NP)
            pe_in = np.empty((P, D, 2 * (K + NP)), bf16)
            for q in range(P):
                ch = cbp[q].astype(bf16)
                cl = (cbp[q] - ch.astype(np.float32)).astype(bf16)
                zh = zfp[q].astype(bf16)
                zl = (zfp[q] - zh.astype(np.float32)).astype(bf16)
                pe_in[q, :, :K] = ch
                pe_in[q, :, K : 2 * K] = cl
                pe_in[q, :, 2 * K : 2 * K + NP] = zh
                pe_in[q, :, 2 * K + NP :] = zl
        elif dot_mode in ("bf16x2", "bf16full"):
            ch = cbt.astype(bf16)
            cl = (cbt - ch.astype(np.float32)).astype(bf16)
            zh = zft.astype(bf16)
            zl = (zft - zh.astype(np.float32)).astype(bf16)
            pe_in = np.ascontiguousarray(np.concatenate([ch, cl, zh, zl], axis=1))
        else:
            pe_in = np.ascontiguousarray(
                np.concatenate([cbt, zft], axis=1), dtype=np.float32
            )

        if dot_mode == "bf16full":
            # exact 3-way bf16 split of zn; cn truncated to bf16 hi part
            s1 = zn_s.astype(bf16)
            r1 = zn_s - s1.astype(np.float32)
            s2 = r1.astype(bf16)
            r2 = r1 - s2.astype(np.float32)
            s3 = r2.astype(bf16)
            assert float(np.abs(
                zn_s - (s1.astype(np.float32) + s2.astype(np.float32)
                        + s3.astype(np.float32))
            ).max()) == 0.0, "zn bf16 3-split not exact"
            bias2 = np.zeros((P, 4, K + NP), bf16)
            bias2[:, 0, :K] = 1.0
            bias2[:, 1, :K] = 1.0
            bias2[:, 2, :K] = 1.0
            bias2[:, 3, :K] = (-cn).astype(bf16)
            bias2[:, 0, K:] = -s1
            bias2[:, 1, K:] = -s2
            bias2[:, 2, K:] = -s3
            bias2[:, 3, K:] = 1.0
        else:
            bias2 = np.empty((P, 2, K + NP), np.float32)
            bias2[:, 0, :K] = -cn
            bias2[:, 1, :K] = 1.0
            bias2[:, 0, K:] = 1.0
            bias2[:, 1, K:] = -zn_s

        in_maps.append(
            {
                "pe_in": pe_in,
                "zf_n": zf_s,
                "bias2": bias2,
                "zn2": np.ascontiguousarray(
                    (2.0 * zn_s).reshape(P, NT, 128).transpose(0, 2, 1)
                ),
                "cbr": cbr,
            }
        )
    return in_maps, zf, cb


DOT_MODE = "bf16full"
SPLIT_PRELOAD = True


def kernel(z, codebooks):
    from concourse.bass_utils import run_bass_kernel_spmd

    if "nc" not in _CACHE:
        _CACHE["nc"] = _build_nc(dot_mode=DOT_MODE, split_preload=SPLIT_PRELOAD)
    nc = _CACHE["nc"]

    in_maps, zf, cb = _prep_inputs(z, codebooks, dot_mode=DOT_MODE)
    res = run_bass_kernel_spmd(nc, in_maps, core_ids=list(range(NCORES)))
    outs = res.results

    # --- distance_prob: (N, P*K) ---
    distance_prob = np.empty((N, P * K), np.float32)
    zq_pnd = np.empty((P, N, D), np.float32)
    sse_total = 0.0
    for c in range(NCORES):
        lo = c * NSH
        o = outs[c]
        pr = o["prob"]                                # (P, NP, K)
        zqc = o["zq"]                                 # (P, NP, D)
        for p in range(P):
            distance_prob[lo : lo + NSH, p * K : (p + 1) * K] = pr[p, :NSH]
        zq_pnd[:, lo : lo + NSH] = zqc[:, :NSH]
        s = o["sse"].astype(np.float64)               # (128, P*NT)
        # tail tile (i == NT-1) only has rows NSH - 12*128 = 32 valid
        valid_tail = NSH - (NT - 1) * 128
        for p in range(P):
            sse_total += s[:, p * NT : p * NT + NT - 1].sum()
            sse_total += s[:valid_tail, p * NT + NT - 1].sum()

    q_loss = np.float32((1.0 + BETA) * sse_total / (P * N * D))

    # --- zq back to (B, C, H, W) ---
    zq_out = (
        zq_pnd.reshape(P, B, H * W, D)
        .transpose(1, 0, 3, 2)
        .reshape(B, C, H, W)
    )
    return np.ascontiguousarray(zq_out), q_loss, distance_prob
